# revision 16
# baseline (speedup 1.0000x reference)
"""Trainium2 Bass kernel for the NLNN (non-local neural network) block.

Reference semantics (per batch b, with X = x[b] as [1024, 2304] and N = 48*48):
    T   = w1 @ X            [512, 2304]
    PHI = w2 @ X            [512, 2304]
    G   = w3 @ X            [512, 2304]
    T'  = reshape(T,  [2304, 512])   (raw row-major memory reinterpretation)
    G'  = reshape(G,  [2304, 512])
    A   = softmax(T' @ PHI, axis=-1) [2304, 2304]
    Y   = A @ G'            [2304, 512]
    Yr  = reshape(Y, [512, 2304])
    out = X + w4 @ Yr + b4  [1024, 2304]

Sharding: pure data parallelism — batch B=8 mapped 1:1 onto 8 NeuronCores.

On-chip strategy (per core):
  - All matmuls in bf16 (1 PE cycle/row), fp32 PSUM accumulation.
  - The two awkward 4.5-ratio reshapes (T->T', Y->Yr) are realized by
    round-tripping flat buffers through HBM with natural/contiguous access
    patterns; T' is additionally transposed into T'^T (contraction layout)
    with the DMA xbar transpose (bf16).
  - att^T is computed m-major ([m partitions, n free]) so its exp can be
    consumed directly as the stationary operand of the Y matmul.
  - Softmax denominator comes for free: a ones-column is prepended to G'
    so the Y matmul accumulates sum_m exp(att^T[m, n]) in PSUM column 0.
  - Softmax needs no max subtraction: |logits| < ~60 here, exp stays well
    inside fp32/bf16 range.
  - The residual is applied from the resident bf16 xb tile (no second fp32
    copy of x ever touches the device): phase E reads nothing from HBM and
    its DMA budget is pure output writes.
  - Head: w1/xb-chunk-0 are loaded in k-paired slices so the first conv
    matmul only gates on ~512 KB, not 2 MB.
  - Phase E runs chunk 0 for all 8 output row-tiles with only the rt0-2
    Yr contributions first, so the PE has ~5 us of work while the last
    y->Yr HBM round-trip is still in flight.
"""

import numpy as np
import ml_dtypes

import concourse.bass as bass
import concourse.bacc as bacc
import concourse.mybir as mybir
import concourse.tile as tile
from concourse.bass_utils import run_bass_kernel_spmd

F32 = mybir.dt.float32
BF16 = mybir.dt.bfloat16
AF = mybir.ActivationFunctionType
ALU = mybir.AluOpType

C_IN = 1024
C_MID = 512
H = W = 48
N = H * W  # 2304
B = 8
NCORES = 8
KT = C_IN // 128   # 8  k tiles over input channels
MT = C_MID // 128  # 4  tiles over mid channels
NT = N // 128      # 18 tiles over spatial dim
# free-dim chunks of <=512 (one fp32 PSUM bank)
NCHUNKS = [(i, min(i + 512, N)) for i in range(0, N, 512)]


def _emit(nc, tc, t_in, t_out):
    with (
        tc.tile_pool(name="mega", bufs=1) as mega,
        tc.tile_pool(name="psum", bufs=8, space="PSUM") as psp,
        tc.tile_pool(name="dram", bufs=1, space="DRAM") as dramp,
        tc.tile_pool(name="small", bufs=4) as smallp,
    ):
        # ---- long-lived tiles (slots are re-tagged across phases) ----
        phi = mega.tile([128, MT, N], BF16, tag="phi")
        ttT = mega.tile([128, MT, N], BF16, tag="ttT")       # T'^T, [c, n]
        gaug = mega.tile([128, NT, 513], BF16, tag="gaug")   # [ones | G'] per m-tile
        w4s = mega.tile([128, MT, C_IN], BF16, tag="w4s")
        b4s = mega.tile([128, KT], F32, tag="b4s")
        bsml = mega.tile([128, 3 * MT], F32, tag="bsml")     # b1|b2|b3 as [128, 4] each

        # flat HBM intermediates implementing the raw reshapes
        t_dram = dramp.tile([C_MID * N], BF16, tag="t_dram")
        g_dram = dramp.tile([C_MID * N], BF16, tag="g_dram")
        y_dram = dramp.tile([C_MID * N], BF16, tag="y_dram")
        t_w = t_dram[:].rearrange("(t p m) -> p t m", p=128, m=N)
        t_r = t_dram[:].rearrange("(n c) -> n c", c=C_MID)  # T' view [2304, 512]
        g_w = g_dram[:].rearrange("(t p m) -> p t m", p=128, m=N)
        g_r = g_dram[:].rearrange("(t p c) -> p t c", p=128, c=C_MID)  # G' tiles
        y_w = y_dram[:].rearrange("(n c) -> n c", c=C_MID)  # write view [2304, 512]
        y_r = y_dram[:].rearrange("(t p m) -> p t m", p=128, m=N)      # Yr tiles

        # head loads: inputs are host-pre-tiled to [128, ...] row-major so
        # every DMA is fully contiguous on both sides. w1 and xb chunk 0 are
        # loaded in k-paired slices so the first matmul gates on ~512 KB.
        w1s = mega.tile([128, KT, C_MID], BF16, tag="w1s")
        # xb is chunk-major: [128, (chunk | k | m)] contiguous per chunk;
        # it stays resident the whole kernel (it is also the residual source).
        xb = mega.tile([128, KT * N], BF16, tag="xmem")

        def xbr(ci, k):
            n0, n1 = NCHUNKS[ci]
            return xb[:, KT * n0 + k * (n1 - n0):KT * n0 + (k + 1) * (n1 - n0)]

        def load_xb(ci):
            n0, n1 = NCHUNKS[ci]
            nc.sync.dma_start(xb[:, KT * n0:KT * n1], t_in["xb"][:, KT * n0:KT * n1])

        w1t_r = t_in["w1t"][:].rearrange("p (t c) -> p t c", c=C_MID)
        for kp in range(0, KT, 2):
            nc.scalar.dma_start(w1s[:, kp:kp + 2, :], w1t_r[:, kp:kp + 2, :])
            nc.sync.dma_start(xb[:, kp * 512:(kp + 2) * 512],
                              t_in["xb"][:, kp * 512:(kp + 2) * 512])
        # biases come host-packed: single contiguous descriptor per partition
        nc.sync.dma_start(bsml[:], t_in["bs"][:])
        load_xb(1)
        w2s = mega.tile([128, KT, C_MID], BF16, tag="w2s")
        nc.sync.dma_start(w2s[:], t_in["w2t"][:].rearrange("p (t c) -> p t c", c=C_MID))
        load_xb(2)
        load_xb(3)
        w3s = mega.tile([128, KT, C_MID], BF16, tag="w3s")
        nc.sync.dma_start(w3s[:], t_in["w3t"][:].rearrange("p (t c) -> p t c", c=C_MID))
        load_xb(4)

        def conv(ws, boff, dest_sb):
            """dest = w.T @ xb (+bias); chunk-outer so chunk c only needs
            xb chunk c. dest_sb is a [128, MT, N] staging tensor."""
            for ci, (n0, n1) in enumerate(NCHUNKS):
                for mb in range(MT):
                    ps = psp.tile([128, n1 - n0], F32, tag="ps")
                    for k in range(KT):
                        nc.tensor.matmul(
                            ps[:],
                            lhsT=ws[:, k, mb * 128:(mb + 1) * 128],
                            rhs=xbr(ci, k),
                            start=(k == 0),
                            stop=(k == KT - 1),
                        )
                    nc.scalar.activation(dest_sb[:, mb, n0:n1], ps[:],
                                         AF.Identity,
                                         bias=bsml[:, boff * MT + mb:boff * MT + mb + 1])

        # theta conv first: its HBM round trip overlaps phi/g convs
        tstg = mega.tile([128, MT, N], BF16, tag="ae", bufs=2, name="tstg")
        conv(w1s, 0, tstg)
        nc.sync.dma_start(t_w, tstg[:])
        # T'^T via xbar transpose reads of the flat T buffer
        for ct in range(MT):
            nc.sync.dma_start(
                ttT[:, ct, :], t_r[:, ct * 128:(ct + 1) * 128], transpose=True
            )
        conv(w2s, 1, phi)
        nc.vector.memset(gaug[:, :, 0:1], 1.0)
        gstg = mega.tile([128, MT, N], BF16, tag="ae", bufs=2, name="gstg")
        conv(w3s, 2, gstg)
        # write g + read G' in halves so the first half streams while the
        # second half of the conv still computes
        nc.sync.dma_start(g_w[:, 0:2, :], gstg[:, 0:2, :])
        nc.sync.dma_start(gaug[:, 0:9, 1:513], g_r[:, 0:9, :])
        nc.sync.dma_start(g_w[:, 2:4, :], gstg[:, 2:4, :])
        nc.sync.dma_start(gaug[:, 9:18, 1:513], g_r[:, 9:18, :])

        # phase-E constants
        nc.sync.dma_start(w4s[:], t_in["w4t"][:].rearrange("p (t c) -> p t c", c=C_IN))
        nc.sync.dma_start(b4s[:], t_in["b4s"][:])

        # Yr row-tiles reuse the w1-w3 slots (dead after the convs).
        # Their loads are emitted inside the strip loop right after the
        # strip that finishes their source rows — HWDGE dispatch is FIFO
        # in emission order, so emitting them later would queue them
        # behind all remaining y writes.
        yrs = [
            mega.tile([128, N], BF16, tag=("w1s", "w2s", "w3s", "phi")[rt],
                      name="yr_t")
            for rt in range(MT)
        ]

        # ---- attention + Y, strip by strip over n ----
        for si, (n0, n1) in enumerate(NCHUNKS):
            wn = n1 - n0
            ae = mega.tile([128, NT, wn], BF16, tag="ae", bufs=2, name="ae")
            for mb in range(NT):
                ps = psp.tile([128, wn], F32, tag="ps")
                for ct in range(MT):
                    nc.tensor.matmul(
                        ps[:],
                        lhsT=phi[:, ct, mb * 128:(mb + 1) * 128],
                        rhs=ttT[:, ct, n0:n1],
                        start=(ct == 0),
                        stop=(ct == MT - 1),
                    )
                nc.scalar.activation(ae[:, mb, :], ps[:], AF.Exp)
            for nbl in range(wn // 128):
                psA = psp.tile([128, 257], F32, tag="ps")
                psB = psp.tile([128, 256], F32, tag="ps")
                for mt in range(NT):
                    lhs = ae[:, mt, nbl * 128:(nbl + 1) * 128]
                    nc.tensor.matmul(psA[:], lhsT=lhs, rhs=gaug[:, mt, 0:257],
                                     start=(mt == 0), stop=(mt == NT - 1))
                    nc.tensor.matmul(psB[:], lhsT=lhs, rhs=gaug[:, mt, 257:513],
                                     start=(mt == 0), stop=(mt == NT - 1))
                rcp = smallp.tile([128, 1], F32, tag="rcp")
                nc.vector.reciprocal(rcp[:], psA[:, 0:1])
                y_t = smallp.tile([128, C_MID], BF16, tag="yt")
                nc.vector.tensor_scalar_mul(y_t[:, 0:256], psA[:, 1:257], rcp[:])
                nc.vector.tensor_scalar_mul(y_t[:, 256:512], psB[:], rcp[:])
                ng = n0 // 128 + nbl
                nc.sync.dma_start(y_w[ng * 128:(ng + 1) * 128, :], y_t[:])
            if 1 <= si <= 3:
                rt = si - 1
                nc.sync.dma_start(yrs[rt][:], y_r[:, rt, :])

        # ---- final conv + residual: out = x + w4 @ Yr + b4 ----
        # Phase E-0: chunk 0 for ALL cb with only the rt0-2 contributions —
        # ~5 us of PE work that overlaps the yrs[3] round-trip (which gates
        # on the very last y writes). 8 psum banks held simultaneously.
        nc.sync.dma_start(yrs[3][:, 0:512], y_r[:, 3, 0:512])
        nc.sync.dma_start(yrs[3][:, 512:N], y_r[:, 3, 512:N])
        ps0 = []
        for cb in range(KT):
            ps = psp.tile([128, 512], F32, tag="ps", name="ps0")
            for rt in range(3):
                nc.tensor.matmul(
                    ps[:],
                    lhsT=w4s[:, rt, cb * 128:(cb + 1) * 128],
                    rhs=yrs[rt][:, 0:512],
                    start=(rt == 0), stop=False,
                )
            ps0.append(ps)
        # Phase E-1: finish chunk 0 with rt3, add bias+residual, write out.
        for cb in range(KT):
            nc.tensor.matmul(
                ps0[cb][:],
                lhsT=w4s[:, 3, cb * 128:(cb + 1) * 128],
                rhs=yrs[3][:, 0:512],
                start=False, stop=True,
            )
            o0 = smallp.tile([128, 512], BF16, tag="o0")
            if cb % 2:
                nc.vector.scalar_tensor_tensor(
                    o0[:], ps0[cb][:], b4s[:, cb:cb + 1],
                    xbr(0, cb), op0=ALU.add, op1=ALU.add,
                )
            else:
                o1 = smallp.tile([128, 512], BF16, tag="o1", name="o1")
                nc.scalar.activation(o1[:], ps0[cb][:], AF.Identity,
                                     bias=b4s[:, cb:cb + 1])
                nc.vector.tensor_tensor(o0[:], o1[:], xbr(0, cb), op=ALU.add)
            (nc.sync, nc.scalar)[cb % 2].dma_start(
                t_out[cb * 128:(cb + 1) * 128, 0:512], o0[:])
        # Phase E-2: chunks 1-4, cb-outer, each piece written directly so
        # the write queue drains alongside compute; odd-cb writes ride the
        # scalar DMA queue (their adds never touch the scalar engine).
        for cb in range(KT):
            for ci, (n0, n1) in enumerate(NCHUNKS[1:], start=1):
                ps = psp.tile([128, n1 - n0], F32, tag="ps", name="ps")
                for rt in range(MT):
                    nc.tensor.matmul(
                        ps[:],
                        lhsT=w4s[:, rt, cb * 128:(cb + 1) * 128],
                        rhs=yrs[rt][:, n0:n1],
                        start=(rt == 0),
                        stop=(rt == MT - 1),
                    )
                o = smallp.tile([128, n1 - n0], BF16, tag="o0", name="o")
                if cb % 2:
                    nc.vector.scalar_tensor_tensor(
                        o[:], ps[:], b4s[:, cb:cb + 1],
                        xbr(ci, cb), op0=ALU.add, op1=ALU.add,
                    )
                else:
                    o1 = smallp.tile([128, n1 - n0], BF16, tag="o1", name="o1")
                    nc.scalar.activation(o1[:], ps[:], AF.Identity,
                                         bias=b4s[:, cb:cb + 1])
                    nc.vector.tensor_tensor(o[:], o1[:], xbr(ci, cb), op=ALU.add)
                (nc.sync, nc.scalar)[cb % 2].dma_start(
                    t_out[cb * 128:(cb + 1) * 128, n0:n1], o[:])


def build_module():
    nc = bacc.Bacc("TRN2", target_bir_lowering=False, debug=False)
    t_in = {
        "xb": nc.dram_tensor("xb", [128, KT * N], BF16, kind="ExternalInput").ap(),
        "w1t": nc.dram_tensor("w1t", [128, KT * C_MID], BF16, kind="ExternalInput").ap(),
        "w2t": nc.dram_tensor("w2t", [128, KT * C_MID], BF16, kind="ExternalInput").ap(),
        "w3t": nc.dram_tensor("w3t", [128, KT * C_MID], BF16, kind="ExternalInput").ap(),
        "w4t": nc.dram_tensor("w4t", [128, MT * C_IN], BF16, kind="ExternalInput").ap(),
        "bs": nc.dram_tensor("bs", [128, 3 * MT], F32, kind="ExternalInput").ap(),
        "b4s": nc.dram_tensor("b4s", [128, KT], F32, kind="ExternalInput").ap(),
    }
    t_out = nc.dram_tensor("out", [C_IN, N], BF16, kind="ExternalOutput").ap()
    with tile.TileContext(nc) as tc:
        _emit(nc, tc, t_in, t_out)
    nc.compile()
    return nc


_NC = None


def _get_nc():
    global _NC
    if _NC is None:
        _NC = build_module()
    return _NC


def _ptile(a):
    """[T*128, C] -> [128, T*C] with the 128-partition dim outermost."""
    t = a.shape[0] // 128
    return np.ascontiguousarray(
        a.reshape(t, 128, a.shape[1]).transpose(1, 0, 2).reshape(128, -1)
    )


def make_in_maps(x, w1, b1, w2, b2, w3, b3, w4, b4):
    bf = ml_dtypes.bfloat16
    bs = np.concatenate(
        [np.asarray(b, np.float32).reshape(MT, 128).T for b in (b1, b2, b3)], axis=1)
    shared = {
        "w1t": _ptile(np.asarray(w1, np.float32).T).astype(bf),
        "w2t": _ptile(np.asarray(w2, np.float32).T).astype(bf),
        "w3t": _ptile(np.asarray(w3, np.float32).T).astype(bf),
        "w4t": _ptile(np.asarray(w4, np.float32).T).astype(bf),
        "bs": np.ascontiguousarray(bs),
        "b4s": np.ascontiguousarray(np.asarray(b4, np.float32).reshape(KT, 128).T),
    }
    x = np.asarray(x, np.float32)
    maps = []
    for i in range(B):
        xi = x[i].reshape(C_IN, N)
        x8 = xi.reshape(KT, 128, N)
        xbt = np.concatenate(
            [x8[:, :, n0:n1].transpose(1, 0, 2).reshape(128, -1)
             for (n0, n1) in NCHUNKS], axis=1)
        maps.append({"xb": np.ascontiguousarray(xbt).astype(bf), **shared})
    return maps


def _run(in_maps, **kw):
    return run_bass_kernel_spmd(_get_nc(), in_maps, list(range(NCORES)), **kw)


def kernel(x, w1, b1, w2, b2, w3, b3, w4, b4):
    res = _run(make_in_maps(x, w1, b1, w2, b2, w3, b3, w4, b4))
    out = np.stack([np.asarray(res.results[i]["out"]) for i in range(B)])
    return out.reshape(B, C_IN, H, W).astype(np.float32)


# revision 18
# speedup vs baseline: 1.0099x; 1.0099x over previous
"""Trainium2 Bass kernel for the NLNN (non-local neural network) block.

Reference semantics (per batch b, with X = x[b] as [1024, 2304] and N = 48*48):
    T   = w1 @ X            [512, 2304]
    PHI = w2 @ X            [512, 2304]
    G   = w3 @ X            [512, 2304]
    T'  = reshape(T,  [2304, 512])   (raw row-major memory reinterpretation)
    G'  = reshape(G,  [2304, 512])
    A   = softmax(T' @ PHI, axis=-1) [2304, 2304]
    Y   = A @ G'            [2304, 512]
    Yr  = reshape(Y, [512, 2304])
    out = X + w4 @ Yr + b4  [1024, 2304]

Sharding: pure data parallelism — batch B=8 mapped 1:1 onto 8 NeuronCores.

On-chip strategy (per core):
  - All matmuls in bf16 (1 PE cycle/row), fp32 PSUM accumulation.
  - The two awkward 4.5-ratio reshapes (T->T', Y->Yr) are realized by
    round-tripping flat buffers through HBM with natural/contiguous access
    patterns; T' is additionally transposed into T'^T (contraction layout)
    with the DMA xbar transpose (bf16).
  - att^T is computed m-major ([m partitions, n free]) so its exp can be
    consumed directly as the stationary operand of the Y matmul.
  - Softmax denominator comes for free: a ones-column is prepended to G'
    so the Y matmul accumulates sum_m exp(att^T[m, n]) in PSUM column 0.
  - Softmax needs no max subtraction: |logits| < ~60 here, exp stays well
    inside fp32/bf16 range.
  - The residual is applied from the resident bf16 xb tile (no second fp32
    copy of x ever touches the device): phase E reads nothing from HBM and
    its DMA budget is pure output writes.
  - Head: w1/xb-chunk-0 are loaded in k-paired slices so the first conv
    matmul only gates on ~512 KB, not 2 MB.
  - Phase E runs chunk 0 for all 8 output row-tiles with only the rt0-2
    Yr contributions first, so the PE has ~5 us of work while the last
    y->Yr HBM round-trip is still in flight.
"""

import numpy as np
import ml_dtypes

import concourse.bass as bass
import concourse.bacc as bacc
import concourse.mybir as mybir
import concourse.tile as tile
from concourse.bass_utils import run_bass_kernel_spmd

F32 = mybir.dt.float32
BF16 = mybir.dt.bfloat16
AF = mybir.ActivationFunctionType
ALU = mybir.AluOpType

C_IN = 1024
C_MID = 512
H = W = 48
N = H * W  # 2304
B = 8
NCORES = 8
KT = C_IN // 128   # 8  k tiles over input channels
MT = C_MID // 128  # 4  tiles over mid channels
NT = N // 128      # 18 tiles over spatial dim
# free-dim chunks of <=512 (one fp32 PSUM bank)
NCHUNKS = [(i, min(i + 512, N)) for i in range(0, N, 512)]


def _emit(nc, tc, t_in, t_out):
    with (
        tc.tile_pool(name="mega", bufs=1) as mega,
        tc.tile_pool(name="psum", bufs=8, space="PSUM") as psp,
        tc.tile_pool(name="dram", bufs=1, space="DRAM") as dramp,
        tc.tile_pool(name="small", bufs=4) as smallp,
    ):
        # ---- long-lived tiles (slots are re-tagged across phases) ----
        phi = mega.tile([128, MT, N], BF16, tag="phi")
        ttT = mega.tile([128, MT, N], BF16, tag="ttT")       # T'^T, [c, n]
        gaug = mega.tile([128, NT, 513], BF16, tag="gaug")   # [ones | G'] per m-tile
        w4s = mega.tile([128, MT, C_IN], BF16, tag="w4s")
        b4s = mega.tile([128, KT], F32, tag="b4s")
        bsml = mega.tile([128, 3 * MT], F32, tag="bsml")     # b1|b2|b3 as [128, 4] each

        # flat HBM intermediates implementing the raw reshapes
        t_dram = dramp.tile([C_MID * N], BF16, tag="t_dram")
        g_dram = dramp.tile([C_MID * N], BF16, tag="g_dram")
        y_dram = dramp.tile([C_MID * N], BF16, tag="y_dram")
        t_w = t_dram[:].rearrange("(t p m) -> p t m", p=128, m=N)
        t_r = t_dram[:].rearrange("(n c) -> n c", c=C_MID)  # T' view [2304, 512]
        g_w = g_dram[:].rearrange("(t p m) -> p t m", p=128, m=N)
        g_r = g_dram[:].rearrange("(t p c) -> p t c", p=128, c=C_MID)  # G' tiles
        y_w = y_dram[:].rearrange("(n c) -> n c", c=C_MID)  # write view [2304, 512]
        y_r = y_dram[:].rearrange("(t p m) -> p t m", p=128, m=N)      # Yr tiles

        # head loads: inputs are host-pre-tiled to [128, ...] row-major so
        # every DMA is fully contiguous on both sides. w1 and xb chunk 0 are
        # loaded in k-paired slices so the first matmul gates on ~512 KB.
        w1s = mega.tile([128, KT, C_MID], BF16, tag="w1s")
        # xb is chunk-major: [128, (chunk | k | m)] contiguous per chunk;
        # it stays resident the whole kernel (it is also the residual source).
        xb = mega.tile([128, KT * N], BF16, tag="xmem")

        def xbr(ci, k):
            n0, n1 = NCHUNKS[ci]
            return xb[:, KT * n0 + k * (n1 - n0):KT * n0 + (k + 1) * (n1 - n0)]

        def load_xb(ci):
            n0, n1 = NCHUNKS[ci]
            nc.sync.dma_start(xb[:, KT * n0:KT * n1], t_in["xb"][:, KT * n0:KT * n1])

        w1t_r = t_in["w1t"][:].rearrange("p (t c) -> p t c", c=C_MID)
        nc.scalar.dma_start(w1s[:, 0:1, :], w1t_r[:, 0:1, :])
        nc.sync.dma_start(xb[:, 0:512], t_in["xb"][:, 0:512])
        nc.scalar.dma_start(w1s[:, 1:2, :], w1t_r[:, 1:2, :])
        nc.sync.dma_start(xb[:, 512:1024], t_in["xb"][:, 512:1024])
        for kp in range(2, KT, 2):
            nc.scalar.dma_start(w1s[:, kp:kp + 2, :], w1t_r[:, kp:kp + 2, :])
            nc.sync.dma_start(xb[:, kp * 512:(kp + 2) * 512],
                              t_in["xb"][:, kp * 512:(kp + 2) * 512])
        # biases come host-packed: single contiguous descriptor per partition
        nc.sync.dma_start(bsml[:], t_in["bs"][:])
        load_xb(1)
        w2s = mega.tile([128, KT, C_MID], BF16, tag="w2s")
        nc.sync.dma_start(w2s[:], t_in["w2t"][:].rearrange("p (t c) -> p t c", c=C_MID))
        load_xb(2)
        load_xb(3)
        w3s = mega.tile([128, KT, C_MID], BF16, tag="w3s")
        nc.sync.dma_start(w3s[:], t_in["w3t"][:].rearrange("p (t c) -> p t c", c=C_MID))
        load_xb(4)

        def conv(ws, boff, dest_sb):
            """dest = w.T @ xb (+bias); chunk-outer so chunk c only needs
            xb chunk c. dest_sb is a [128, MT, N] staging tensor."""
            for ci, (n0, n1) in enumerate(NCHUNKS):
                for mb in range(MT):
                    ps = psp.tile([128, n1 - n0], F32, tag="ps")
                    for k in range(KT):
                        nc.tensor.matmul(
                            ps[:],
                            lhsT=ws[:, k, mb * 128:(mb + 1) * 128],
                            rhs=xbr(ci, k),
                            start=(k == 0),
                            stop=(k == KT - 1),
                        )
                    nc.scalar.activation(dest_sb[:, mb, n0:n1], ps[:],
                                         AF.Identity,
                                         bias=bsml[:, boff * MT + mb:boff * MT + mb + 1])

        # theta conv first: its HBM round trip overlaps phi/g convs
        tstg = mega.tile([128, MT, N], BF16, tag="ae", bufs=2, name="tstg")
        conv(w1s, 0, tstg)
        nc.sync.dma_start(t_w, tstg[:])
        # T'^T via xbar transpose reads of the flat T buffer
        for ct in range(MT):
            nc.sync.dma_start(
                ttT[:, ct, :], t_r[:, ct * 128:(ct + 1) * 128], transpose=True
            )
        conv(w2s, 1, phi)
        nc.vector.memset(gaug[:, :, 0:1], 1.0)
        gstg = mega.tile([128, MT, N], BF16, tag="ae", bufs=2, name="gstg")
        conv(w3s, 2, gstg)
        # write g + read G' in halves so the first half streams while the
        # second half of the conv still computes
        nc.sync.dma_start(g_w[:, 0:2, :], gstg[:, 0:2, :])
        nc.sync.dma_start(gaug[:, 0:9, 1:513], g_r[:, 0:9, :])
        nc.sync.dma_start(g_w[:, 2:4, :], gstg[:, 2:4, :])
        nc.sync.dma_start(gaug[:, 9:18, 1:513], g_r[:, 9:18, :])

        # phase-E constants
        nc.sync.dma_start(w4s[:], t_in["w4t"][:].rearrange("p (t c) -> p t c", c=C_IN))
        nc.sync.dma_start(b4s[:], t_in["b4s"][:])

        # Yr row-tiles reuse the w1-w3 slots (dead after the convs).
        # Their loads are emitted inside the strip loop right after the
        # strip that finishes their source rows — HWDGE dispatch is FIFO
        # in emission order, so emitting them later would queue them
        # behind all remaining y writes.
        yrs = [
            mega.tile([128, N], BF16, tag=("w1s", "w2s", "w3s", "phi")[rt],
                      name="yr_t")
            for rt in range(MT)
        ]

        # ---- attention + Y, strip by strip over n ----
        for si, (n0, n1) in enumerate(NCHUNKS):
            wn = n1 - n0
            ae = mega.tile([128, NT, wn], BF16, tag="ae", bufs=2, name="ae")
            for mb in range(NT):
                ps = psp.tile([128, wn], F32, tag="ps")
                for ct in range(MT):
                    nc.tensor.matmul(
                        ps[:],
                        lhsT=phi[:, ct, mb * 128:(mb + 1) * 128],
                        rhs=ttT[:, ct, n0:n1],
                        start=(ct == 0),
                        stop=(ct == MT - 1),
                    )
                nc.scalar.activation(ae[:, mb, :], ps[:], AF.Exp)
            for nbl in range(wn // 128):
                psA = psp.tile([128, 257], F32, tag="ps")
                psB = psp.tile([128, 256], F32, tag="ps")
                for mt in range(NT):
                    lhs = ae[:, mt, nbl * 128:(nbl + 1) * 128]
                    nc.tensor.matmul(psA[:], lhsT=lhs, rhs=gaug[:, mt, 0:257],
                                     start=(mt == 0), stop=(mt == NT - 1))
                    nc.tensor.matmul(psB[:], lhsT=lhs, rhs=gaug[:, mt, 257:513],
                                     start=(mt == 0), stop=(mt == NT - 1))
                rcp = smallp.tile([128, 1], F32, tag="rcp")
                nc.vector.reciprocal(rcp[:], psA[:, 0:1])
                y_t = smallp.tile([128, C_MID], BF16, tag="yt")
                nc.vector.tensor_scalar_mul(y_t[:, 0:256], psA[:, 1:257], rcp[:])
                nc.vector.tensor_scalar_mul(y_t[:, 256:512], psB[:], rcp[:])
                ng = n0 // 128 + nbl
                nc.sync.dma_start(y_w[ng * 128:(ng + 1) * 128, :], y_t[:])
            if 1 <= si <= 3:
                rt = si - 1
                nc.sync.dma_start(yrs[rt][:], y_r[:, rt, :])

        # ---- final conv + residual: out = x + w4 @ Yr + b4 ----
        # Phase E-0: chunk 0 for ALL cb with only the rt0-2 contributions —
        # ~5 us of PE work that overlaps the yrs[3] round-trip (which gates
        # on the very last y writes). 8 psum banks held simultaneously.
        nc.sync.dma_start(yrs[3][:, 0:512], y_r[:, 3, 0:512])
        nc.sync.dma_start(yrs[3][:, 512:N], y_r[:, 3, 512:N])
        ps0 = []
        for cb in range(KT):
            ps = psp.tile([128, 512], F32, tag="ps", name="ps0")
            for rt in range(3):
                nc.tensor.matmul(
                    ps[:],
                    lhsT=w4s[:, rt, cb * 128:(cb + 1) * 128],
                    rhs=yrs[rt][:, 0:512],
                    start=(rt == 0), stop=False,
                )
            ps0.append(ps)
        # Phase E-1: finish chunk 0 with rt3, add bias+residual, write out.
        for cb in range(KT):
            nc.tensor.matmul(
                ps0[cb][:],
                lhsT=w4s[:, 3, cb * 128:(cb + 1) * 128],
                rhs=yrs[3][:, 0:512],
                start=False, stop=True,
            )
            o0 = smallp.tile([128, 512], BF16, tag="o0")
            if cb % 2:
                nc.vector.scalar_tensor_tensor(
                    o0[:], ps0[cb][:], b4s[:, cb:cb + 1],
                    xbr(0, cb), op0=ALU.add, op1=ALU.add,
                )
            else:
                o1 = smallp.tile([128, 512], BF16, tag="o1", name="o1")
                nc.scalar.activation(o1[:], ps0[cb][:], AF.Identity,
                                     bias=b4s[:, cb:cb + 1])
                nc.vector.tensor_tensor(o0[:], o1[:], xbr(0, cb), op=ALU.add)
            nc.sync.dma_start(t_out[cb * 128:(cb + 1) * 128, 0:512], o0[:])
        # Phase E-2: chunks 1-4, cb-outer with staging + 2-piece flush.
        for cb in range(KT):
            out_t = mega.tile([128, N - 512], BF16,
                              tag=("ttT" if cb % 2 == 0 else "gaug"), name="out_t")
            for ci, (n0, n1) in enumerate(NCHUNKS[1:], start=1):
                ps = psp.tile([128, n1 - n0], F32, tag="ps", name="ps")
                for rt in range(MT):
                    nc.tensor.matmul(
                        ps[:],
                        lhsT=w4s[:, rt, cb * 128:(cb + 1) * 128],
                        rhs=yrs[rt][:, n0:n1],
                        start=(rt == 0),
                        stop=(rt == MT - 1),
                    )
                if cb % 2:
                    nc.vector.scalar_tensor_tensor(
                        out_t[:, n0 - 512:n1 - 512], ps[:], b4s[:, cb:cb + 1],
                        xbr(ci, cb), op0=ALU.add, op1=ALU.add,
                    )
                else:
                    o1 = smallp.tile([128, n1 - n0], BF16, tag="o1", name="o1")
                    nc.scalar.activation(o1[:], ps[:], AF.Identity,
                                         bias=b4s[:, cb:cb + 1])
                    nc.vector.tensor_tensor(out_t[:, n0 - 512:n1 - 512],
                                            o1[:], xbr(ci, cb), op=ALU.add)
                # flush progressively so the final write is small
                if n1 == 1536:
                    nc.sync.dma_start(t_out[cb * 128:(cb + 1) * 128, 512:1536],
                                      out_t[:, 0:1024])
                elif n1 == 2048:
                    nc.sync.dma_start(t_out[cb * 128:(cb + 1) * 128, 1536:2048],
                                      out_t[:, 1024:1536])
            nc.sync.dma_start(t_out[cb * 128:(cb + 1) * 128, 2048:N],
                              out_t[:, 1536:N - 512])


def build_module():
    nc = bacc.Bacc("TRN2", target_bir_lowering=False, debug=False)
    t_in = {
        "xb": nc.dram_tensor("xb", [128, KT * N], BF16, kind="ExternalInput").ap(),
        "w1t": nc.dram_tensor("w1t", [128, KT * C_MID], BF16, kind="ExternalInput").ap(),
        "w2t": nc.dram_tensor("w2t", [128, KT * C_MID], BF16, kind="ExternalInput").ap(),
        "w3t": nc.dram_tensor("w3t", [128, KT * C_MID], BF16, kind="ExternalInput").ap(),
        "w4t": nc.dram_tensor("w4t", [128, MT * C_IN], BF16, kind="ExternalInput").ap(),
        "bs": nc.dram_tensor("bs", [128, 3 * MT], F32, kind="ExternalInput").ap(),
        "b4s": nc.dram_tensor("b4s", [128, KT], F32, kind="ExternalInput").ap(),
    }
    t_out = nc.dram_tensor("out", [C_IN, N], BF16, kind="ExternalOutput").ap()
    with tile.TileContext(nc) as tc:
        _emit(nc, tc, t_in, t_out)
    nc.compile()
    return nc


_NC = None


def _get_nc():
    global _NC
    if _NC is None:
        _NC = build_module()
    return _NC


def _ptile(a):
    """[T*128, C] -> [128, T*C] with the 128-partition dim outermost."""
    t = a.shape[0] // 128
    return np.ascontiguousarray(
        a.reshape(t, 128, a.shape[1]).transpose(1, 0, 2).reshape(128, -1)
    )


def make_in_maps(x, w1, b1, w2, b2, w3, b3, w4, b4):
    bf = ml_dtypes.bfloat16
    bs = np.concatenate(
        [np.asarray(b, np.float32).reshape(MT, 128).T for b in (b1, b2, b3)], axis=1)
    shared = {
        "w1t": _ptile(np.asarray(w1, np.float32).T).astype(bf),
        "w2t": _ptile(np.asarray(w2, np.float32).T).astype(bf),
        "w3t": _ptile(np.asarray(w3, np.float32).T).astype(bf),
        "w4t": _ptile(np.asarray(w4, np.float32).T).astype(bf),
        "bs": np.ascontiguousarray(bs),
        "b4s": np.ascontiguousarray(np.asarray(b4, np.float32).reshape(KT, 128).T),
    }
    x = np.asarray(x, np.float32)
    maps = []
    for i in range(B):
        xi = x[i].reshape(C_IN, N)
        x8 = xi.reshape(KT, 128, N)
        xbt = np.concatenate(
            [x8[:, :, n0:n1].transpose(1, 0, 2).reshape(128, -1)
             for (n0, n1) in NCHUNKS], axis=1)
        maps.append({"xb": np.ascontiguousarray(xbt).astype(bf), **shared})
    return maps


def _run(in_maps, **kw):
    return run_bass_kernel_spmd(_get_nc(), in_maps, list(range(NCORES)), **kw)


def kernel(x, w1, b1, w2, b2, w3, b3, w4, b4):
    res = _run(make_in_maps(x, w1, b1, w2, b2, w3, b3, w4, b4))
    out = np.stack([np.asarray(res.results[i]["out"]) for i in range(B)])
    return out.reshape(B, C_IN, H, W).astype(np.float32)
